# revision 25
# baseline (speedup 1.0000x reference)
"""Trainium2 Bass kernel for a GAT block.

Math (after algebraic simplification of the reference):
  h[b,f,n,k] = x[b,:,f,n] @ W[:,k] + bW[k]
  s2[b,f,n]  = h[b,f,n,:] @ a2 = v.x + const   (s1/ab/const cancel in softmax)
  d[b,f,n]   = softmax_n(s2)[n] * mask[n,n]
  out[b,k,f,n] = d[b,f,n] * h[b,f,n,k] = sum_c W[c,k] (x*d)[c,f,n] + bW[k] d[f,n]

Sharding: data-parallel over batch, 4 batches per core on 8 cores.

Layout: one batch (2048 frames) = 4 interleaved q-units. SBUF partition
32c+s holds frames [64s, 64s+64) of channel c (3.2KB DMA descriptors);
q-unit g covers frames {64s+16g ..+16} = column slice [400g, 400g+400)
of the batch tile. fsub s of unit g = 16 frames.

Per batch: s2/softmax for all 4 units fused at [128(=32 fsub x 4 units),
400]: 4 col-tiled vsel matmuls -> one psum bank; exp/rowsum/recip/
mask-mul/renorm once per batch (amortized 4x).

Per q-unit:
  pdd [128, 400] = rep4_rep[32g:32g+32].T @ dd4[32g:32g+32] (PE);
  x4s = x4 * pdd (DVE, bf16): x*d rows 0:96, d rows 96:128;
  16 matmuls, stationary wsel[tp] [128,128] bf16 (FWL), psum rows
  (2k+jj), pairs in 2-bank psum tiles [128, 1024];
  evictions: 7 strided pair-copies (3 DVE / 4 ACT) + last pair split
  across both engines; one store per batch (osb [128, 25600], 51.2KB
  descriptors).

A ~10-matmul warm-up at kernel start flips the PE HAM clock gate to
8/8 before the first unit. Output bf16, upcast to fp32 on host
(rel err ~8e-3 « 2e-2 tolerance).
"""

import sys

if "/opt/trn_rl_repo" not in sys.path:
    sys.path.insert(0, "/opt/trn_rl_repo")

import numpy as np
import ml_dtypes

B, C, F, N, H = 32, 3, 2048, 25, 64
NCORES = 8
BPC = B // NCORES   # batches per core
G = 4               # interleaved q-units per batch
QF = F // G         # 512 frames per q-unit
FSUB = 16           # frames per fsub row (per unit)
NS = QF // FSUB     # 32 fsub rows
FN = F * N
TW = FSUB * N       # 400, columns per unit tile
BW = G * TW         # 1600, columns per batch tile
NT = NS // 2        # 16 matmuls (of 32 frames) per q-unit
NG = NT // 2        # 8 psum tile-pairs per q-unit

# full-pair evictions routed to DVE; SPLIT_TG is halved across engines
DVE_TGS = (1, 4, 6)
SPLIT_TG = 3

_NC_CACHE = {}


def _build_nc():
    import concourse.bass as bass
    import concourse.bacc as bacc
    import concourse.tile as tile
    from concourse import mybir

    f32 = mybir.dt.float32
    bf16 = mybir.dt.bfloat16
    MULT = mybir.AluOpType.mult
    AX = mybir.AxisListType.X
    EXP = mybir.ActivationFunctionType.Exp

    nc = bacc.Bacc()
    x_d = nc.declare_dram_parameter("x", [BPC, C, F, N], bf16, isOutput=False)
    wsel_d = nc.declare_dram_parameter("wsel", [128, NT, 128], bf16, isOutput=False)
    rep4_d = nc.declare_dram_parameter("rep4", [128, 128], bf16, isOutput=False)
    vsel_d = nc.declare_dram_parameter("vsel", [128, NS], bf16, isOutput=False)
    md_d = nc.declare_dram_parameter("md400", [128, TW], bf16, isOutput=False)
    out_d = nc.declare_dram_parameter("out", [BPC, H, F, N], bf16, isOutput=True)

    with tile.TileContext(nc) as tc:
        with (
            tc.tile_pool(name="singles", bufs=1) as singles,
            tc.tile_pool(name="x16", bufs=2) as x16_pool,
            tc.tile_pool(name="sm", bufs=2) as sm_pool,
            tc.tile_pool(name="x4s", bufs=2) as x4s_pool,
            tc.tile_pool(name="osb", bufs=2) as osb_pool,
            tc.tile_pool(name="ps", bufs=4, space="PSUM") as ps_pool,
        ):
            wsel_sb = singles.tile([128, NT, 128], bf16)
            nc.sync.dma_start(out=wsel_sb[:], in_=wsel_d[:, :, :])
            rep4_sb = singles.tile([128, 128], bf16)
            nc.sync.dma_start(out=rep4_sb[:], in_=rep4_d[:, :])
            vsel_sb = singles.tile([128, NS], bf16)
            nc.sync.dma_start(out=vsel_sb[:], in_=vsel_d[:, :])
            md_sb = singles.tile([128, TW], bf16)
            nc.sync.dma_start(out=md_sb[:], in_=md_d[:, :])

            NU = BPC * G        # 16 q-units per core
            nload = [0]

            def emit_load(b):
                """x16 [128, 1600] bf16 for batch b: rows 0:96 from HBM."""
                base = x_d[b, :, 0:1, :]  # for offset only
                x16 = x16_pool.tile([128, BW], bf16, tag="x16")
                # rows 96:128 are only ever written here; with a 2-deep pool
                # it suffices to initialize each buffer once
                if nload[0] < 2:
                    nc.vector.memset(x16[96:128, :], 1.0)
                nload[0] += 1
                src = bass.AP(
                    tensor=base.tensor,
                    offset=base.offset,
                    ap=[[FN, C], [BW, NS], [1, BW]],
                )
                nc.sync.dma_start(out=x16[0:96, :], in_=src)
                return x16

            def x4_view(x16, g):
                return x16[:, g * TW : (g + 1) * TW]

            def emit_s2_mm(x16):
                """4 col-tiled vsel matmuls -> s2p [128, 400] (a ps slot)."""
                s2t = ps_pool.tile([128, 1024], f32, tag="ph")
                s2p = s2t[:, 0:TW]
                for g in range(G):
                    nc.tensor.matmul(
                        s2t[32 * g : 32 * (g + 1), 0:TW],
                        vsel_sb[:],
                        x4_view(x16, g),
                        start=True,
                        stop=True,
                        tile_position=(0, 32 * g),
                    )
                return s2p

            def emit_s2_exp(s2p):
                """exp leads the ACT queue (PSUM -> SBUF bf16)."""
                e = sm_pool.tile([128, TW], bf16, tag="e")
                nc.scalar.activation(out=e[:], in_=s2p, func=EXP)
                return e

            def emit_s2_chain(e):
                """Fused softmax for a whole batch -> dd4 [128, 400] bf16."""
                ev = e[:].rearrange("p (a b) -> p a b", b=N)
                z = sm_pool.tile([128, FSUB], f32, tag="z")
                nc.vector.reduce_sum(out=z[:], in_=ev, axis=AX)
                r = sm_pool.tile([128, FSUB], f32, tag="r")
                nc.vector.reciprocal(out=r[:], in_=z[:])
                em = sm_pool.tile([128, TW], bf16, tag="em")
                nc.vector.tensor_tensor(out=em[:], in0=e[:], in1=md_sb[:], op=MULT)
                dd4 = sm_pool.tile([128, TW], bf16, tag="dd4")
                rr = r[:, :]
                r_bc = bass.AP(
                    tensor=rr.tensor,
                    offset=rr.offset,
                    ap=[rr.ap[0], [1, FSUB], [0, N]],
                )
                nc.vector.tensor_tensor(out=dd4[:], in0=em[:], in1=r_bc, op=MULT)
                return dd4

            def emit_scale(ui):
                """pdd = rep4.T @ dd4-slice (PE); x4s = x4 * pdd (DVE)."""
                b, g = divmod(ui, G)
                dd4 = dd4_t[b]
                pdt = ps_pool.tile([128, 1024], f32, tag="ph")
                pdd = pdt[:, 0:TW]
                nc.tensor.matmul(
                    pdd[:, :],
                    rep4_sb[32 * g : 32 * (g + 1), :],
                    dd4[32 * g : 32 * (g + 1), :],
                    start=True,
                    stop=True,
                    tile_position=(32 * g, 0),
                )
                x4s = x4s_pool.tile([128, TW], bf16, tag="x4s")
                nc.vector.tensor_tensor(
                    out=x4s[:], in0=x4_view(x16_t[b], g), in1=pdd[:], op=MULT
                )
                return x4s

            # Software pipeline, steady state at iteration ui = (b, g):
            #   PE : rep4(ui+1), wsel(ui) x16, [g==1: vsel x4 (b+1)]
            #   DVE: x4s(ui+1), [g==2: chain(b+1)], evictions(ui)
            #   ACT: [g==2: exp(b+1)], evictions(ui)
            # dd4(b+1) is ready one unit before rep4 of batch b+1 needs it;
            # every PE op's inputs exist at iteration start, so the PE never
            # blocks mid-iteration and HAM stays warm.
            x16_t = [None] * (BPC + 1)
            dd4_t = [None] * (BPC + 1)
            s2_t = [None] * (BPC + 1)
            x4s_t = [None] * NU
            x16_t[0] = emit_load(0)

            # PE warm-up: ~10 back-to-back matmuls flip the HAM clock gate
            # to 8/8 before the first real unit (outputs are discarded).
            for w in range(10):
                phw = ps_pool.tile([128, 1024], f32, tag="ph")
                nc.tensor.matmul(
                    phw[:, 0:512],
                    wsel_sb[:, w % NT, :],
                    wsel_sb[:].rearrange("p a b -> p (a b)")[:, 0:512],
                    start=True,
                    stop=True,
                )

            s2_t[0] = emit_s2_mm(x16_t[0])
            e_t = [None] * (BPC + 1)
            e_t[0] = emit_s2_exp(s2_t[0])
            dd4_t[0] = emit_s2_chain(e_t[0])
            x4s_t[0] = emit_scale(0)
            osb = None

            for ui in range(NU):
                b, g = divmod(ui, G)
                if g == 0:
                    osb = osb_pool.tile([128, G * NT, TW], bf16)
                    if b + 1 <= BPC - 1:
                        x16_t[b + 1] = emit_load(b + 1)
                if ui + 1 < NU:
                    x4s_t[ui + 1] = emit_scale(ui + 1)
                if g == 2 and b + 1 <= BPC - 1:
                    # exp leads the ACT queue this iteration
                    e_t[b + 1] = emit_s2_exp(s2_t[b + 1])
                x4s = x4s_t[ui]
                # ---- 16 matmuls in 2-bank pairs + evictions
                osv = osb[:, :, :]
                for tg in range(NG):
                    ph = ps_pool.tile([128, 1024], f32, tag="ph")
                    nc.tensor.matmul(
                        ph[:, 0:TW],
                        wsel_sb[:, 2 * tg, :],
                        x4s[:, :],
                        start=True,
                        stop=True,
                    )
                    nc.tensor.matmul(
                        ph[:, 512 : 512 + TW],
                        wsel_sb[:, 2 * tg + 1, :],
                        x4s[:, :],
                        start=True,
                        stop=True,
                    )
                    phv = ph[:, :]
                    if tg == SPLIT_TG:
                        # halve this mid-block pair across both engines
                        dstA = bass.AP(
                            tensor=osv.tensor,
                            offset=osv.offset + (8 * tg + g) * TW,
                            ap=[osv.ap[0], [1, 1], [1, TW]],
                        )
                        dstB = bass.AP(
                            tensor=osv.tensor,
                            offset=osv.offset + (8 * tg + 4 + g) * TW,
                            ap=[osv.ap[0], [1, 1], [1, TW]],
                        )
                        nc.vector.tensor_copy(dstA, ph[:, 0:TW])
                        nc.scalar.copy(dstB, ph[:, 512 : 512 + TW])
                        continue
                    src = bass.AP(
                        tensor=phv.tensor,
                        offset=phv.offset,
                        ap=[phv.ap[0], [512, 2], [1, TW]],
                    )
                    dst = bass.AP(
                        tensor=osv.tensor,
                        offset=osv.offset + (8 * tg + g) * TW,
                        ap=[osv.ap[0], [G * TW, 2], [1, TW]],
                    )
                    if tg in DVE_TGS:
                        nc.vector.tensor_copy(dst, src)
                    else:
                        nc.scalar.copy(dst, src)
                # vsel matmuls for the next batch go to the PE after this
                # unit's wsel block (the x16 load has had a full unit)
                if g == 1 and b + 1 <= BPC - 1:
                    s2_t[b + 1] = emit_s2_mm(x16_t[b + 1])
                # chain DVE ops at the tail of the g==2 DVE queue so they
                # never block ready evictions (in-order queues)
                if g == 2 and b + 1 <= BPC - 1:
                    dd4_t[b + 1] = emit_s2_chain(e_t[b + 1])
                # ---- one store per batch (sync ring; SP engine is idle)
                if g == G - 1:
                    osl = out_d[b, :, 0:1, :]
                    dst = bass.AP(
                        tensor=osl.tensor,
                        offset=osl.offset,
                        ap=[[FN, H], [G * NT * TW, 2], [1, G * NT * TW]],
                    )
                    nc.sync.dma_start(out=dst, in_=osb[:, :, :])
    nc.compile()
    return nc


def _get_nc():
    if "nc" not in _NC_CACHE:
        _NC_CACHE["nc"] = _build_nc()
    return _NC_CACHE["nc"]


def _make_in_maps(x, mask, W, bW, a1, a2, ab):
    bf = ml_dtypes.bfloat16
    x = np.ascontiguousarray(np.asarray(x, np.float32)).astype(bf)
    mask = np.asarray(mask, np.float32)
    W = np.asarray(W, np.float32)
    bW = np.asarray(bW, np.float32)
    a2 = np.asarray(a2, np.float32)

    v = (W @ a2).astype(np.float32)                    # [C]
    md = np.diag(mask).astype(np.float32)              # [N]

    # wsel[row = 32 c + fsub, tp, col = 2 k + jj]:
    #   delta[fsub == tp + 16 jj] * (W[c, k] if c < 3 else bW[k])
    # (column order (k, jj)-interleaved so the store DMA is affine)
    wsel = np.zeros((128, NT, 128), np.float32)
    cols = np.arange(H)
    for tp in range(NT):
        for jj in range(2):
            fsub = tp + 16 * jj
            for c in range(3):
                wsel[32 * c + fsub, tp, 2 * cols + jj] = W[c]
            wsel[96 + fsub, tp, 2 * cols + jj] = bW
    rep4 = np.tile(np.eye(NS, dtype=np.float32), (G, G))
    vsel = np.zeros((128, NS), np.float32)
    for c in range(3):
        vsel[32 * c : 32 * (c + 1), :] = np.eye(NS, dtype=np.float32) * v[c]
    md400 = np.tile(np.tile(md, FSUB)[None, :], (128, 1)).astype(np.float32)

    wsel = wsel.astype(bf)
    rep4 = rep4.astype(bf)
    vsel = vsel.astype(bf)
    md400 = md400.astype(bf)

    in_maps = []
    for cix in range(NCORES):
        in_maps.append(
            {
                "x": np.ascontiguousarray(x[cix * BPC : (cix + 1) * BPC]),
                "wsel": wsel,
                "rep4": rep4,
                "vsel": vsel,
                "md400": md400,
            }
        )
    return in_maps


def run(x, mask, W, bW, a1, a2, ab, **run_kwargs):
    from concourse.bass_utils import run_bass_kernel_spmd

    nc = _get_nc()
    in_maps = _make_in_maps(x, mask, W, bW, a1, a2, ab)
    res = run_bass_kernel_spmd(nc, in_maps, core_ids=list(range(NCORES)), **run_kwargs)
    out = np.concatenate(
        [np.asarray(res.results[i]["out"]).astype(np.float32) for i in range(NCORES)],
        axis=0,
    )
    return out, res


def kernel(x, mask, W, bW, a1, a2, ab):
    out, _ = run(x, mask, W, bW, a1, a2, ab)
    return out


# revision 30
# speedup vs baseline: 1.0097x; 1.0097x over previous
"""Trainium2 Bass kernel for a GAT block.

Math (after algebraic simplification of the reference):
  h[b,f,n,k] = x[b,:,f,n] @ W[:,k] + bW[k]
  s2[b,f,n]  = h[b,f,n,:] @ a2 = v.x + const   (s1/ab/const cancel in softmax)
  d[b,f,n]   = softmax_n(s2)[n] * mask[n,n]
  out[b,k,f,n] = d[b,f,n] * h[b,f,n,k] = sum_c W[c,k] (x*d)[c,f,n] + bW[k] d[f,n]

Sharding: data-parallel over batch, 4 batches per core on 8 cores.

Layout: one batch (2048 frames) = 4 interleaved q-units. SBUF partition
32c+s holds frames [64s, 64s+64) of channel c (3.2KB DMA descriptors);
q-unit g covers frames {64s+16g ..+16} = column slice [400g, 400g+400)
of the batch tile. fsub s of unit g = 16 frames.

Per batch: s2/softmax for all 4 units fused at [128(=32 fsub x 4 units),
400]: 4 col-tiled vsel matmuls -> one psum bank; exp/rowsum/recip/
mask-mul/renorm once per batch (amortized 4x).

Per q-unit:
  pdd [128, 400] = rep4_rep[32g:32g+32].T @ dd4[32g:32g+32] (PE);
  x4s = x4 * pdd (DVE, bf16): x*d rows 0:96, d rows 96:128;
  16 matmuls, stationary wsel[tp] [128,128] bf16 (FWL), psum rows
  (2k+jj), pairs in 2-bank psum tiles [128, 1024];
  evictions: 7 strided pair-copies (3 DVE / 4 ACT) + last pair split
  across both engines; one store per batch (osb [128, 25600], 51.2KB
  descriptors).

A ~10-matmul warm-up at kernel start flips the PE HAM clock gate to
8/8 before the first unit. Output bf16, upcast to fp32 on host
(rel err ~8e-3 « 2e-2 tolerance).
"""

import sys

if "/opt/trn_rl_repo" not in sys.path:
    sys.path.insert(0, "/opt/trn_rl_repo")

import numpy as np
import ml_dtypes

B, C, F, N, H = 32, 3, 2048, 25, 64
NCORES = 8
BPC = B // NCORES   # batches per core
G = 4               # interleaved q-units per batch
QF = F // G         # 512 frames per q-unit
FSUB = 16           # frames per fsub row (per unit)
NS = QF // FSUB     # 32 fsub rows
FN = F * N
TW = FSUB * N       # 400, columns per unit tile
BW = G * TW         # 1600, columns per batch tile
NT = NS // 2        # 16 matmuls (of 32 frames) per q-unit
NG = NT // 2        # 8 psum tile-pairs per q-unit

# full-pair evictions routed to DVE; SPLIT_TG (if >= 0) is halved across
# both engines
DVE_TGS = (1, 4, 6)
SPLIT_TG = -1

_NC_CACHE = {}


def _build_nc():
    import concourse.bass as bass
    import concourse.bacc as bacc
    import concourse.tile as tile
    from concourse import mybir

    f32 = mybir.dt.float32
    bf16 = mybir.dt.bfloat16
    MULT = mybir.AluOpType.mult
    AX = mybir.AxisListType.X
    EXP = mybir.ActivationFunctionType.Exp

    nc = bacc.Bacc()
    x_d = nc.declare_dram_parameter("x", [BPC, C, F, N], bf16, isOutput=False)
    wsel_d = nc.declare_dram_parameter("wsel", [128, NT, 128], bf16, isOutput=False)
    rep4_d = nc.declare_dram_parameter("rep4", [128, 128], bf16, isOutput=False)
    vsel_d = nc.declare_dram_parameter("vsel", [128, NS], bf16, isOutput=False)
    md_d = nc.declare_dram_parameter("md400", [128, TW], bf16, isOutput=False)
    out_d = nc.declare_dram_parameter("out", [BPC, H, F, N], bf16, isOutput=True)

    with tile.TileContext(nc) as tc:
        with (
            tc.tile_pool(name="singles", bufs=1) as singles,
            tc.tile_pool(name="x16", bufs=2) as x16_pool,
            tc.tile_pool(name="sm", bufs=2) as sm_pool,
            tc.tile_pool(name="x4s", bufs=2) as x4s_pool,
            tc.tile_pool(name="osb", bufs=2) as osb_pool,
            tc.tile_pool(name="ps", bufs=3, space="PSUM") as ps_pool,
            tc.tile_pool(name="psd", bufs=1, space="PSUM") as psd_pool,
            tc.tile_pool(name="pss", bufs=1, space="PSUM") as pss_pool,
        ):
            wsel_sb = singles.tile([128, NT, 128], bf16)
            nc.sync.dma_start(out=wsel_sb[:], in_=wsel_d[:, :, :])
            rep4_sb = singles.tile([128, 128], bf16)
            nc.sync.dma_start(out=rep4_sb[:], in_=rep4_d[:, :])
            vsel_sb = singles.tile([128, NS], bf16)
            nc.sync.dma_start(out=vsel_sb[:], in_=vsel_d[:, :])
            md_sb = singles.tile([128, TW], bf16)
            nc.sync.dma_start(out=md_sb[:], in_=md_d[:, :])

            NU = BPC * G        # 16 q-units per core
            nload = [0]

            def emit_load(b):
                """x16 [128, 1600] bf16 for batch b: rows 0:96 from HBM."""
                base = x_d[b, :, 0:1, :]  # for offset only
                x16 = x16_pool.tile([128, BW], bf16, tag="x16")
                # rows 96:128 are only ever written here; with a 2-deep pool
                # it suffices to initialize each buffer once
                if nload[0] < 2:
                    nc.vector.memset(x16[96:128, :], 1.0)
                nload[0] += 1
                src = bass.AP(
                    tensor=base.tensor,
                    offset=base.offset,
                    ap=[[FN, C], [BW, NS], [1, BW]],
                )
                nc.sync.dma_start(out=x16[0:96, :], in_=src)
                return x16

            def x4_view(x16, g):
                return x16[:, g * TW : (g + 1) * TW]

            def emit_s2_mm(x16):
                """4 col-tiled vsel matmuls -> s2p [128, 400] (one bank)."""
                s2t = pss_pool.tile([128, TW], f32, tag="s2p")
                s2p = s2t[:, :]
                for g in range(G):
                    nc.tensor.matmul(
                        s2t[32 * g : 32 * (g + 1), :],
                        vsel_sb[:],
                        x4_view(x16, g),
                        start=True,
                        stop=True,
                        tile_position=(0, 32 * g),
                    )
                return s2p

            def emit_s2_exp(s2p):
                """exp leads the ACT queue (PSUM -> SBUF bf16)."""
                e = sm_pool.tile([128, TW], bf16, tag="e")
                nc.scalar.activation(out=e[:], in_=s2p, func=EXP)
                return e

            def emit_s2_chain(e):
                """Fused softmax for a whole batch -> dd4 [128, 400] bf16."""
                ev = e[:].rearrange("p (a b) -> p a b", b=N)
                z = sm_pool.tile([128, FSUB], f32, tag="z")
                nc.vector.reduce_sum(out=z[:], in_=ev, axis=AX)
                r = sm_pool.tile([128, FSUB], f32, tag="r")
                nc.vector.reciprocal(out=r[:], in_=z[:])
                em = sm_pool.tile([128, TW], bf16, tag="em")
                nc.vector.tensor_tensor(out=em[:], in0=e[:], in1=md_sb[:], op=MULT)
                dd4 = sm_pool.tile([128, TW], bf16, tag="dd4")
                rr = r[:, :]
                r_bc = bass.AP(
                    tensor=rr.tensor,
                    offset=rr.offset,
                    ap=[rr.ap[0], [1, FSUB], [0, N]],
                )
                nc.vector.tensor_tensor(out=dd4[:], in0=em[:], in1=r_bc, op=MULT)
                return dd4

            def emit_scale(ui):
                """pdd = rep4.T @ dd4-slice (PE); x4s = x4 * pdd (DVE)."""
                b, g = divmod(ui, G)
                dd4 = dd4_t[b]
                pdt = psd_pool.tile([128, TW], f32, tag="pdd")
                nc.tensor.matmul(
                    pdt[:, :],
                    rep4_sb[32 * g : 32 * (g + 1), :],
                    dd4[32 * g : 32 * (g + 1), :],
                    start=True,
                    stop=True,
                    tile_position=(32 * g, 0),
                )
                x4s = x4s_pool.tile([128, TW], bf16, tag="x4s")
                nc.vector.tensor_tensor(
                    out=x4s[:], in0=x4_view(x16_t[b], g), in1=pdt[:], op=MULT
                )
                return x4s

            # Software pipeline, steady state at iteration ui = (b, g):
            #   PE : rep4(ui+1), wsel(ui) x16, [g==1: vsel x4 (b+1)]
            #   DVE: x4s(ui+1), [g==2: chain(b+1)], evictions(ui)
            #   ACT: [g==2: exp(b+1)], evictions(ui)
            # dd4(b+1) is ready one unit before rep4 of batch b+1 needs it;
            # every PE op's inputs exist at iteration start, so the PE never
            # blocks mid-iteration and HAM stays warm.
            x16_t = [None] * (BPC + 1)
            dd4_t = [None] * (BPC + 1)
            s2_t = [None] * (BPC + 1)
            x4s_t = [None] * NU
            x16_t[0] = emit_load(0)

            # PE warm-up: ~10 back-to-back matmuls flip the HAM clock gate
            # to 8/8 before the first real unit (outputs are discarded).
            for w in range(10):
                phw = ps_pool.tile([128, 1024], f32, tag="ph")
                nc.tensor.matmul(
                    phw[:, 0:512],
                    wsel_sb[:, w % NT, :],
                    wsel_sb[:].rearrange("p a b -> p (a b)")[:, 0:512],
                    start=True,
                    stop=True,
                )

            s2_t[0] = emit_s2_mm(x16_t[0])
            e_t = [None] * (BPC + 1)
            e_t[0] = emit_s2_exp(s2_t[0])
            dd4_t[0] = emit_s2_chain(e_t[0])
            x4s_t[0] = emit_scale(0)
            osb = None

            for ui in range(NU):
                b, g = divmod(ui, G)
                if g == 0:
                    osb = osb_pool.tile([128, G * NT, TW], bf16)
                    if b + 1 <= BPC - 1:
                        x16_t[b + 1] = emit_load(b + 1)
                if ui + 1 < NU:
                    x4s_t[ui + 1] = emit_scale(ui + 1)
                if g == 2 and b + 1 <= BPC - 1:
                    # exp leads the ACT queue this iteration
                    e_t[b + 1] = emit_s2_exp(s2_t[b + 1])
                x4s = x4s_t[ui]
                # ---- 16 matmuls in 2-bank pairs + evictions
                osv = osb[:, :, :]
                for tg in range(NG):
                    ph = ps_pool.tile([128, 1024], f32, tag="ph")
                    nc.tensor.matmul(
                        ph[:, 0:TW],
                        wsel_sb[:, 2 * tg, :],
                        x4s[:, :],
                        start=True,
                        stop=True,
                    )
                    nc.tensor.matmul(
                        ph[:, 512 : 512 + TW],
                        wsel_sb[:, 2 * tg + 1, :],
                        x4s[:, :],
                        start=True,
                        stop=True,
                    )
                    phv = ph[:, :]
                    if tg == SPLIT_TG:
                        # halve this mid-block pair across both engines
                        dstA = bass.AP(
                            tensor=osv.tensor,
                            offset=osv.offset + (8 * tg + g) * TW,
                            ap=[osv.ap[0], [1, 1], [1, TW]],
                        )
                        dstB = bass.AP(
                            tensor=osv.tensor,
                            offset=osv.offset + (8 * tg + 4 + g) * TW,
                            ap=[osv.ap[0], [1, 1], [1, TW]],
                        )
                        nc.vector.tensor_copy(dstA, ph[:, 0:TW])
                        nc.scalar.copy(dstB, ph[:, 512 : 512 + TW])
                        continue
                    src = bass.AP(
                        tensor=phv.tensor,
                        offset=phv.offset,
                        ap=[phv.ap[0], [512, 2], [1, TW]],
                    )
                    dst = bass.AP(
                        tensor=osv.tensor,
                        offset=osv.offset + (8 * tg + g) * TW,
                        ap=[osv.ap[0], [G * TW, 2], [1, TW]],
                    )
                    if tg in DVE_TGS:
                        nc.vector.tensor_copy(dst, src)
                    else:
                        nc.scalar.copy(dst, src)
                # vsel matmuls for the next batch go to the PE after this
                # unit's wsel block (the x16 load has had a full unit)
                if g == 1 and b + 1 <= BPC - 1:
                    s2_t[b + 1] = emit_s2_mm(x16_t[b + 1])
                # chain DVE ops at the tail of the g==2 DVE queue so they
                # never block ready evictions (in-order queues)
                if g == 2 and b + 1 <= BPC - 1:
                    dd4_t[b + 1] = emit_s2_chain(e_t[b + 1])
                # ---- one store per batch (sync ring; SP engine is idle)
                if g == G - 1:
                    osl = out_d[b, :, 0:1, :]
                    dst = bass.AP(
                        tensor=osl.tensor,
                        offset=osl.offset,
                        ap=[[FN, H], [G * NT * TW, 2], [1, G * NT * TW]],
                    )
                    nc.sync.dma_start(out=dst, in_=osb[:, :, :])
    nc.compile()
    return nc


def _get_nc():
    if "nc" not in _NC_CACHE:
        _NC_CACHE["nc"] = _build_nc()
    return _NC_CACHE["nc"]


def _make_in_maps(x, mask, W, bW, a1, a2, ab):
    bf = ml_dtypes.bfloat16
    x = np.ascontiguousarray(np.asarray(x, np.float32)).astype(bf)
    mask = np.asarray(mask, np.float32)
    W = np.asarray(W, np.float32)
    bW = np.asarray(bW, np.float32)
    a2 = np.asarray(a2, np.float32)

    v = (W @ a2).astype(np.float32)                    # [C]
    md = np.diag(mask).astype(np.float32)              # [N]

    # wsel[row = 32 c + fsub, tp, col = 2 k + jj]:
    #   delta[fsub == tp + 16 jj] * (W[c, k] if c < 3 else bW[k])
    # (column order (k, jj)-interleaved so the store DMA is affine)
    wsel = np.zeros((128, NT, 128), np.float32)
    cols = np.arange(H)
    for tp in range(NT):
        for jj in range(2):
            fsub = tp + 16 * jj
            for c in range(3):
                wsel[32 * c + fsub, tp, 2 * cols + jj] = W[c]
            wsel[96 + fsub, tp, 2 * cols + jj] = bW
    rep4 = np.tile(np.eye(NS, dtype=np.float32), (G, G))
    vsel = np.zeros((128, NS), np.float32)
    for c in range(3):
        vsel[32 * c : 32 * (c + 1), :] = np.eye(NS, dtype=np.float32) * v[c]
    md400 = np.tile(np.tile(md, FSUB)[None, :], (128, 1)).astype(np.float32)

    wsel = wsel.astype(bf)
    rep4 = rep4.astype(bf)
    vsel = vsel.astype(bf)
    md400 = md400.astype(bf)

    in_maps = []
    for cix in range(NCORES):
        in_maps.append(
            {
                "x": np.ascontiguousarray(x[cix * BPC : (cix + 1) * BPC]),
                "wsel": wsel,
                "rep4": rep4,
                "vsel": vsel,
                "md400": md400,
            }
        )
    return in_maps


def run(x, mask, W, bW, a1, a2, ab, **run_kwargs):
    from concourse.bass_utils import run_bass_kernel_spmd

    nc = _get_nc()
    in_maps = _make_in_maps(x, mask, W, bW, a1, a2, ab)
    res = run_bass_kernel_spmd(nc, in_maps, core_ids=list(range(NCORES)), **run_kwargs)
    out = np.concatenate(
        [np.asarray(res.results[i]["out"]).astype(np.float32) for i in range(NCORES)],
        axis=0,
    )
    return out, res


def kernel(x, mask, W, bW, a1, a2, ab):
    out, _ = run(x, mask, W, bW, a1, a2, ab)
    return out


# revision 31
# speedup vs baseline: 1.0733x; 1.0630x over previous
"""Trainium2 Bass kernel for a GAT block.

Math (after algebraic simplification of the reference):
  h[b,f,n,k] = x[b,:,f,n] @ W[:,k] + bW[k]
  s2[b,f,n]  = h[b,f,n,:] @ a2 = v.x + const   (s1/ab/const cancel in softmax)
  d[b,f,n]   = softmax_n(s2)[n] * mask[n,n]
  out[b,k,f,n] = d[b,f,n] * h[b,f,n,k] = sum_c W[c,k] (x*d)[c,f,n] + bW[k] d[f,n]

Sharding: data-parallel over batch, 4 batches per core on 8 cores.

Layout: one batch (2048 frames) = 4 interleaved q-units. SBUF partition
32c+s holds frames [64s, 64s+64) of channel c (3.2KB DMA descriptors);
q-unit g covers frames {64s+16g .. 64s+16g+16} = column slice
[400g, 400g+400) of the batch tile. fsub s of unit g = 16 frames.

Device pipeline per q-unit, shapes are [partitions, free]:
  1. x16 [128, 1600] bf16 per batch: rows 32c+s = x[c], rows 96:128 = 1.0.
  2. s2p [32, 400] = vsel.T @ x4 on PE (vsel[32c+s, s] = v[c], rows 96+: 0).
  3. softmax: e = exp(s2p) (ACT) -> z = rowsum25 (DVE) -> r = 1/z (DVE)
     -> em = e*md400 (DVE 2x bf16) -> dd32 = em*r_bc (DVE).
  4. pdd [128, 400] = rep4.T @ dd32 (PE); x4s = x4 * pdd (DVE, bf16):
     x*d rows 0:96, d rows 96:128.
  5. 16 matmuls, stationary wsel[tp] [128,128] bf16 (FWL): psum rows
     (2k+jj); pairs share a 2-bank psum tile [128, 1024].
  6. evictions (3 DVE / 5 ACT): strided [128, 2x400] psum -> osb slot
     4*tp+g; one store per batch (osb [128, 25600], 51.2KB descriptors).

Software pipeline, steady state at iteration ui:
  PE : rep4(ui+1), vsel(ui+2), wsel(ui) x16
  DVE: x4s(ui+1), chain(ui+2), evict(ui) x3
  ACT: exp(ui+2), evict(ui) x5
so every PE op's inputs are ready at iteration start. A ~10-matmul
warm-up at kernel start flips the PE HAM clock gate to 8/8 before the
first unit. Output bf16, upcast to fp32 on host (rel err ~8e-3 « 2e-2).
"""

import sys

if "/opt/trn_rl_repo" not in sys.path:
    sys.path.insert(0, "/opt/trn_rl_repo")

import numpy as np
import ml_dtypes

B, C, F, N, H = 32, 3, 2048, 25, 64
NCORES = 8
BPC = B // NCORES   # batches per core
G = 4               # interleaved q-units per batch
QF = F // G         # 512 frames per q-unit
FSUB = 16           # frames per fsub row (per unit)
NS = QF // FSUB     # 32 fsub rows
FN = F * N
TW = FSUB * N       # 400, columns per unit tile
BW = G * TW         # 1600, columns per batch tile
NT = NS // 2        # 16 matmuls (of 32 frames) per q-unit
NG = NT // 2        # 8 psum tile-pairs per q-unit

# evictions (of the 8 tile-pairs per unit) routed to DVE; rest go to ACT
DVE_TGS = (1, 4, 6)

_NC_CACHE = {}


def _build_nc():
    import concourse.bass as bass
    import concourse.bacc as bacc
    import concourse.tile as tile
    from concourse import mybir

    f32 = mybir.dt.float32
    bf16 = mybir.dt.bfloat16
    MULT = mybir.AluOpType.mult
    AX = mybir.AxisListType.X
    EXP = mybir.ActivationFunctionType.Exp

    nc = bacc.Bacc()
    x_d = nc.declare_dram_parameter("x", [BPC, C, F, N], bf16, isOutput=False)
    wsel_d = nc.declare_dram_parameter("wsel", [128, NT, 128], bf16, isOutput=False)
    rep4_d = nc.declare_dram_parameter("rep4", [NS, 128], bf16, isOutput=False)
    vsel_d = nc.declare_dram_parameter("vsel", [128, NS], bf16, isOutput=False)
    md_d = nc.declare_dram_parameter("md400", [NS, TW], bf16, isOutput=False)
    out_d = nc.declare_dram_parameter("out", [BPC, H, F, N], bf16, isOutput=True)

    with tile.TileContext(nc) as tc:
        with (
            tc.tile_pool(name="singles", bufs=1) as singles,
            tc.tile_pool(name="x16", bufs=2) as x16_pool,
            tc.tile_pool(name="sm", bufs=3) as sm_pool,
            tc.tile_pool(name="x4s", bufs=2) as x4s_pool,
            tc.tile_pool(name="osb", bufs=2) as osb_pool,
            tc.tile_pool(name="ps", bufs=3, space="PSUM") as ps_pool,
            tc.tile_pool(name="psd", bufs=1, space="PSUM") as psd_pool,
            tc.tile_pool(name="pss", bufs=1, space="PSUM") as pss_pool,
        ):
            wsel_sb = singles.tile([128, NT, 128], bf16)
            nc.sync.dma_start(out=wsel_sb[:], in_=wsel_d[:, :, :])
            rep4_sb = singles.tile([NS, 128], bf16)
            nc.sync.dma_start(out=rep4_sb[:], in_=rep4_d[:, :])
            vsel_sb = singles.tile([128, NS], bf16)
            nc.sync.dma_start(out=vsel_sb[:], in_=vsel_d[:, :])
            md_sb = singles.tile([NS, TW], bf16)
            nc.sync.dma_start(out=md_sb[:], in_=md_d[:, :])

            NU = BPC * G        # 16 q-units per core
            nload = [0]

            def emit_load(b):
                """x16 [128, 1600] bf16 for batch b: rows 0:96 from HBM."""
                base = x_d[b, :, 0:1, :]  # for offset only
                x16 = x16_pool.tile([128, BW], bf16, tag="x16")
                # rows 96:128 are only ever written here; with a 2-deep pool
                # it suffices to initialize each buffer once
                if nload[0] < 2:
                    nc.vector.memset(x16[96:128, :], 1.0)
                nload[0] += 1
                src = bass.AP(
                    tensor=base.tensor,
                    offset=base.offset,
                    ap=[[FN, C], [BW, NS], [1, BW]],
                )
                nc.sync.dma_start(out=x16[0:96, :], in_=src)
                return x16

            def x4_view(x16, g):
                return x16[:, g * TW : (g + 1) * TW]

            def emit_s2(x16, g):
                """s2 matmul + softmax chain -> dd32 [32, 400] bf16."""
                s2p = pss_pool.tile([NS, TW], f32, tag="s2p")
                nc.tensor.matmul(
                    s2p[:, :], vsel_sb[:], x4_view(x16, g), start=True, stop=True
                )
                e = sm_pool.tile([NS, TW], bf16, tag="e")
                nc.scalar.activation(out=e[:], in_=s2p[:], func=EXP)
                ev = e[:].rearrange("p (a b) -> p a b", b=N)
                z = sm_pool.tile([NS, FSUB], f32, tag="z")
                nc.vector.reduce_sum(out=z[:], in_=ev, axis=AX)
                r = sm_pool.tile([NS, FSUB], f32, tag="r")
                nc.vector.reciprocal(out=r[:], in_=z[:])
                em = sm_pool.tile([NS, TW], bf16, tag="em")
                nc.vector.tensor_tensor(out=em[:], in0=e[:], in1=md_sb[:], op=MULT)
                dd32 = sm_pool.tile([NS, TW], bf16, tag="dd32")
                rr = r[:, :]
                r_bc = bass.AP(
                    tensor=rr.tensor,
                    offset=rr.offset,
                    ap=[rr.ap[0], [1, FSUB], [0, N]],
                )
                nc.vector.tensor_tensor(out=dd32[:], in0=em[:], in1=r_bc, op=MULT)
                return dd32

            def emit_scale(ui):
                """pdd = rep4.T @ dd32 (PE); x4s = x4 * pdd (DVE)."""
                b, g = divmod(ui, G)
                pdd = psd_pool.tile([128, TW], f32, tag="pdd")
                nc.tensor.matmul(
                    pdd[:, :], rep4_sb[:], dd_t[ui][:], start=True, stop=True
                )
                x4s = x4s_pool.tile([128, TW], bf16, tag="x4s")
                nc.vector.tensor_tensor(
                    out=x4s[:], in0=x4_view(x16_t[b], g), in1=pdd[:], op=MULT
                )
                return x4s

            x16_t = [None] * (BPC + 1)
            dd_t = [None] * NU
            x4s_t = [None] * NU
            x16_t[0] = emit_load(0)

            # PE warm-up: ~10 back-to-back matmuls flip the HAM clock gate
            # to 8/8 before the first real unit (outputs are discarded).
            for w in range(10):
                phw = ps_pool.tile([128, 1024], f32, tag="ph")
                nc.tensor.matmul(
                    phw[:, 0:512],
                    wsel_sb[:, w % NT, :],
                    wsel_sb[:].rearrange("p a b -> p (a b)")[:, 0:512],
                    start=True,
                    stop=True,
                )

            dd_t[0] = emit_s2(x16_t[0], 0)
            if NU > 1:
                dd_t[1] = emit_s2(x16_t[0], 1)
            x4s_t[0] = emit_scale(0)
            osb = None

            for ui in range(NU):
                b, g = divmod(ui, G)
                if g == 0:
                    osb = osb_pool.tile([128, G * NT, TW], bf16)
                    if b + 1 <= BPC - 1:
                        x16_t[b + 1] = emit_load(b + 1)
                if ui + 1 < NU:
                    x4s_t[ui + 1] = emit_scale(ui + 1)
                if ui + 2 < NU:
                    bn, gn = divmod(ui + 2, G)
                    dd_t[ui + 2] = emit_s2(x16_t[bn], gn)
                x4s = x4s_t[ui]
                # ---- 16 matmuls in 2-bank pairs + evictions
                osv = osb[:, :, :]
                for tg in range(NG):
                    ph = ps_pool.tile([128, 1024], f32, tag="ph")
                    nc.tensor.matmul(
                        ph[:, 0:TW],
                        wsel_sb[:, 2 * tg, :],
                        x4s[:, :],
                        start=True,
                        stop=True,
                    )
                    nc.tensor.matmul(
                        ph[:, 512 : 512 + TW],
                        wsel_sb[:, 2 * tg + 1, :],
                        x4s[:, :],
                        start=True,
                        stop=True,
                    )
                    phv = ph[:, :]
                    src = bass.AP(
                        tensor=phv.tensor,
                        offset=phv.offset,
                        ap=[phv.ap[0], [512, 2], [1, TW]],
                    )
                    # output slots for (unit g, pair tp=2tg,2tg+1):
                    # 4*tp+g and 4*(tp+1)+g -> stride 4*TW
                    dst = bass.AP(
                        tensor=osv.tensor,
                        offset=osv.offset + (8 * tg + g) * TW,
                        ap=[osv.ap[0], [G * TW, 2], [1, TW]],
                    )
                    if tg in DVE_TGS:
                        nc.vector.tensor_copy(dst, src)
                    else:
                        nc.scalar.copy(dst, src)
                # ---- one store per batch
                if g == G - 1:
                    osl = out_d[b, :, 0:1, :]
                    dst = bass.AP(
                        tensor=osl.tensor,
                        offset=osl.offset,
                        ap=[[FN, H], [G * NT * TW, 2], [1, G * NT * TW]],
                    )
                    eng = nc.sync if b % 2 == 0 else nc.scalar
                    eng.dma_start(out=dst, in_=osb[:, :, :])
    nc.compile()
    return nc


def _get_nc():
    if "nc" not in _NC_CACHE:
        _NC_CACHE["nc"] = _build_nc()
    return _NC_CACHE["nc"]


def _make_in_maps(x, mask, W, bW, a1, a2, ab):
    bf = ml_dtypes.bfloat16
    x = np.ascontiguousarray(np.asarray(x, np.float32)).astype(bf)
    mask = np.asarray(mask, np.float32)
    W = np.asarray(W, np.float32)
    bW = np.asarray(bW, np.float32)
    a2 = np.asarray(a2, np.float32)

    v = (W @ a2).astype(np.float32)                    # [C]
    md = np.diag(mask).astype(np.float32)              # [N]

    # wsel[row = 32 c + fsub, tp, col = 2 k + jj]:
    #   delta[fsub == tp + 16 jj] * (W[c, k] if c < 3 else bW[k])
    # (column order (k, jj)-interleaved so the store DMA is affine)
    wsel = np.zeros((128, NT, 128), np.float32)
    cols = np.arange(H)
    for tp in range(NT):
        for jj in range(2):
            fsub = tp + 16 * jj
            for c in range(3):
                wsel[32 * c + fsub, tp, 2 * cols + jj] = W[c]
            wsel[96 + fsub, tp, 2 * cols + jj] = bW
    rep4 = np.zeros((NS, 128), np.float32)
    for blk in range(4):
        rep4[:, 32 * blk : 32 * (blk + 1)] = np.eye(NS, dtype=np.float32)
    vsel = np.zeros((128, NS), np.float32)
    for c in range(3):
        vsel[32 * c : 32 * (c + 1), :] = np.eye(NS, dtype=np.float32) * v[c]
    md400 = np.tile(np.tile(md, FSUB)[None, :], (NS, 1)).astype(np.float32)

    wsel = wsel.astype(bf)
    rep4 = rep4.astype(bf)
    vsel = vsel.astype(bf)
    md400 = md400.astype(bf)

    in_maps = []
    for cix in range(NCORES):
        in_maps.append(
            {
                "x": np.ascontiguousarray(x[cix * BPC : (cix + 1) * BPC]),
                "wsel": wsel,
                "rep4": rep4,
                "vsel": vsel,
                "md400": md400,
            }
        )
    return in_maps


def run(x, mask, W, bW, a1, a2, ab, **run_kwargs):
    from concourse.bass_utils import run_bass_kernel_spmd

    nc = _get_nc()
    in_maps = _make_in_maps(x, mask, W, bW, a1, a2, ab)
    res = run_bass_kernel_spmd(nc, in_maps, core_ids=list(range(NCORES)), **run_kwargs)
    out = np.concatenate(
        [np.asarray(res.results[i]["out"]).astype(np.float32) for i in range(NCORES)],
        axis=0,
    )
    return out, res


def kernel(x, mask, W, bW, a1, a2, ab):
    out, _ = run(x, mask, W, bW, a1, a2, ab)
    return out
